# revision 20
# baseline (speedup 1.0000x reference)
"""GCN layer (bipartite user-item SpMM x2 + dense + sigmoid) on 8 TRN2 NeuronCores.

Per core k:
  SpMM1 (ue = A @ item_emb), edges sharded by destination user-shard:
    host packs edges into fixed gather calls of TCALL=1024 tokens (the
    dma_gather HW path caps ~1024 indices per instruction); 8 consecutive
    tokens form a chunk with one destination row. SCB=7 calls form a
    scatter group; a destination row appears at most once per group (the
    CCE scatter-add is a read-modify-write, and duplicate rows inside one
    instruction race across the 16 DMA engines; groups are serialized by
    the Tile scheduler's WAW dependency on the accumulator).
    Device pipeline per call: dma_gather rows (bf16) -> scale by edge
    vals (DVE, bf16) -> chunk-sum via matmul with constant block-diagonal
    lhsT (PE, fp32 PSUM) -> stage; per group: dma_scatter_add partials
    into the HBM accumulator (CCE add, fp32).
    Gather tables are range-split to 25000 rows (indices are int16).
  AllGather ue (bf16) across the 8 cores.
  SpMM2 (ie = A^T @ ue): same pipeline, edges sharded by destination item.
  Dense: sigmoid(ue @ u_w), sigmoid(ie @ i_w) via PE transpose + matmul.
"""

import os
import sys
from dataclasses import dataclass

sys.path.insert(0, "/opt/trn_rl_repo")

import numpy as np

import concourse.bacc as bacc
import concourse.bass as bass
import concourse.mybir as mybir
import concourse.tile as tile


@dataclass(frozen=True)
class Cfg:
    U: int = 100000
    I: int = 50000
    D: int = 128
    NCORES: int = 8
    CHUNK: int = 8
    TCALL: int = 896           # tokens per gather call: one call is 57
                               # descriptors/lane, so two gathers + a scatter
                               # pipeline within the 128-desc/lane SWDGE ring
    SCB: int = 6               # calls per scatter group
    RNG1: int = 25000          # item-table range size (SpMM1 gathers)
    RNG2: int = 25000          # user-table range size (SpMM2 gathers)
    G1R: int = 49              # scatter groups per item range (spmm1)
    G2R: int = 25              # scatter groups per user range (spmm2)
    gdt: str = "bf16"          # gather/message dtype: "bf16" or "f32"

    @property
    def USH(self):
        return self.U // self.NCORES

    @property
    def ISH(self):
        return self.I // self.NCORES

    @property
    def NRANGE1(self):
        return (self.I + self.RNG1 - 1) // self.RNG1

    @property
    def NRANGE2(self):
        return (self.U + self.RNG2 - 1) // self.RNG2

    @property
    def NG1(self):
        return self.NRANGE1 * self.G1R      # scatter groups, spmm1

    @property
    def NG2(self):
        return self.NRANGE2 * self.G2R

    @property
    def N1(self):
        return self.NG1 * self.SCB          # gather calls, spmm1

    @property
    def N2(self):
        return self.NG2 * self.SCB

    @property
    def CPC(self):
        return self.TCALL // self.CHUNK     # chunks per call (=128)

    @property
    def CPG(self):
        return self.CPC * self.SCB          # chunk capacity per scatter group

    @property
    def SPG(self):
        return 128 * self.SCB               # scatter tokens per group (padded)

    @property
    def UPAD(self):
        return ((self.USH // 128) + 1) * 128

    @property
    def IPAD(self):
        return ((self.ISH // 128) + 1) * 128


FULL = Cfg()

F32 = mybir.dt.float32
BF16 = mybir.dt.bfloat16
I16 = mybir.dt.int16


# ---------------------------------------------------------------------------
# host-side packing
# ---------------------------------------------------------------------------

def _pack_phase(dst, src, val, dst_sh, src_rng, n_ranges, ngroups_r, cfg, dummy_row):
    """Pack one SpMM phase for one core.

    Returns:
      meta [NCALLS, 128, TCALL//16 + vcols] int16 — per call: gather
        indices (16-wrapped, 8x replicated) then bitcast vals (128-wrapped);
      sidx [NGROUPS, 128, CPG//16] int16 — per scatter group: chunk
        destination rows (16-wrapped, 8x replicated).
    """
    T, C, CPC, CPG, SCB = cfg.TCALL, cfg.CHUNK, cfg.CPC, cfg.CPG, cfg.SCB
    ncalls = n_ranges * ngroups_r * SCB
    ngroups = n_ranges * ngroups_r
    gidx_flat = np.zeros(ncalls * T, np.int16)
    vals_flat = np.zeros(ncalls * T, np.float32)
    SPG = cfg.SPG
    sidx_flat = np.full(ngroups * SPG, dummy_row, np.int16)

    for r in range(n_ranges):
        lo = r * src_rng
        m = (src >= lo) & (src < lo + src_rng)
        d = dst[m]
        s = (src[m] - lo).astype(np.int64)
        v = val[m]
        if d.size == 0:
            continue
        order = np.argsort(d, kind="stable")
        d, s, v = d[order], s[order], v[order]
        cnt = np.bincount(d, minlength=dst_sh)
        q = (cnt + C - 1) // C                      # chunks per dst row
        Q = int(q.sum())
        assert Q <= ngroups_r * CPG, (
            f"range {r}: {Q} chunks exceed budget {ngroups_r * CPG}"
        )
        chunk_off = np.cumsum(q) - q
        chunk_dst = np.repeat(np.arange(dst_sh, dtype=np.int64), q)
        chunk_j = np.arange(Q, dtype=np.int64) - chunk_off[chunk_dst]
        # color chunks -> groups; a dst row appears at most once per group
        color = (chunk_j + chunk_dst) % ngroups_r
        loads = np.bincount(color, minlength=ngroups_r)
        if (loads > CPG).any():
            color = _repair_colors(color, chunk_dst, ngroups_r, CPG)
            loads = np.bincount(color, minlength=ngroups_r)
        assert (loads <= CPG).all()
        order_c = np.argsort(color, kind="stable")
        slot_sorted = np.arange(Q) - (np.cumsum(loads) - loads)[color[order_c]]
        slot = np.empty(Q, np.int64)                # 0..CPG-1 within group
        slot[order_c] = slot_sorted
        grp_glob = r * ngroups_r + color
        call_in_grp = slot // CPC                   # 0..SCB-1
        prow = slot % CPC                           # psum row 0..127
        call_glob = grp_glob * SCB + call_in_grp
        # scatter token order within group: call*128 + psum_row (psum rows
        # CPC..127 are padding that scatters stale data into the dummy row)
        sidx_flat[grp_glob * SPG + call_in_grp * 128 + prow] = (
            chunk_dst.astype(np.int16)
        )
        # gather token position of chunk token t (0..7):
        #   i = (prow//16)*128 + (prow%16)*8 + t  within the call
        tok0 = call_glob * T + (prow // 16) * 128 + (prow % 16) * C
        ecnt_off = np.cumsum(cnt) - cnt
        within = np.arange(d.size) - ecnt_off[d]
        e_chunk = chunk_off[d] + within // C
        e_pos = tok0[e_chunk] + within % C
        gidx_flat[e_pos] = s.astype(np.int16)
        vals_flat[e_pos] = v

    gidx = _wrap16(gidx_flat.reshape(ncalls, T))                 # [NC,128,T/16]
    vnp = mybir.dt.np(BF16 if cfg.gdt == "bf16" else F32)
    vals = (
        vals_flat.reshape(ncalls, T // 128, 128)
        .transpose(0, 2, 1)
        .copy()
        .astype(vnp)
        .view(np.int16)
        .reshape(ncalls, 128, -1)
    )
    meta = np.concatenate([gidx, vals], axis=2)
    sidx = _wrap16(sidx_flat.reshape(ngroups, SPG))              # [NG,128,SPG/16]
    return meta, sidx


def _repair_colors(color, chunk_dst, ncolors, cap):
    color = color.copy()
    loads = np.bincount(color, minlength=ncolors)
    used = [set() for _ in range(ncolors)]
    for i in range(len(color)):
        used[color[i]].add(int(chunk_dst[i]))
    for c in np.where(loads > cap)[0]:
        over = np.where(color == c)[0]
        for i in over[cap:]:
            dsti = int(chunk_dst[i])
            for c2 in np.argsort(loads):
                if loads[c2] < cap and dsti not in used[c2]:
                    used[c].discard(dsti)
                    used[c2].add(dsti)
                    color[i] = c2
                    loads[c] -= 1
                    loads[c2] += 1
                    break
            else:
                raise AssertionError("color repair failed")
    return color


def _wrap16(a2d):
    """[n, m] -> [n, 128, m/16]: token i at partition i%16, slot i//16,
    replicated across the 8 gpsimd 16-partition groups."""
    n, m = a2d.shape
    w = a2d.reshape(n, m // 16, 16).transpose(0, 2, 1)
    return np.tile(w, (1, 8, 1)).copy()


def _chunk_lhst(cfg):
    """8 constant [128,128] matrices: block b maps token e to chunk 16*b+e//8."""
    C = cfg.CHUNK
    per = 128 // C
    lh = np.zeros((8, 128, 128), np.float32)
    for b in range(8):
        for e in range(128):
            lh[b, e, per * b + e // C] = 1.0
    return lh


def prep_inputs(inputs, cfg):
    rows = np.asarray(inputs["edge_rows"]).astype(np.int64)
    cols = np.asarray(inputs["edge_cols"]).astype(np.int64)
    vals = np.asarray(inputs["edge_vals"]).astype(np.float32)
    item = np.ascontiguousarray(np.asarray(inputs["item_embedding"], np.float32))
    u_w = np.ascontiguousarray(np.asarray(inputs["u_w"], np.float32))
    i_w = np.ascontiguousarray(np.asarray(inputs["i_w"], np.float32))
    gnp = mybir.dt.np(BF16 if cfg.gdt == "bf16" else F32)
    shared = {
        "item_tab": item.astype(gnp),
        "u_w": u_w,
        "i_w": i_w,
        "ident": np.eye(128, dtype=np.float32),
        "lhst": _chunk_lhst(cfg).astype(gnp),
    }
    in_maps = []
    ushard = rows // cfg.USH
    ishard = cols // cfg.ISH
    for k in range(cfg.NCORES):
        m1 = ushard == k
        meta1, sidx1 = _pack_phase(
            rows[m1] - k * cfg.USH, cols[m1], vals[m1], cfg.USH,
            cfg.RNG1, cfg.NRANGE1, cfg.G1R, cfg, cfg.UPAD - 1,
        )
        m2 = ishard == k
        meta2, sidx2 = _pack_phase(
            cols[m2] - k * cfg.ISH, rows[m2], vals[m2], cfg.ISH,
            cfg.RNG2, cfg.NRANGE2, cfg.G2R, cfg, cfg.IPAD - 1,
        )
        in_maps.append({
            **shared,
            "g1_meta": meta1, "g1_sidx": sidx1,
            "g2_meta": meta2, "g2_sidx": sidx2,
        })
    return in_maps


# ---------------------------------------------------------------------------
# device program
# ---------------------------------------------------------------------------

def build_program(cfg, phases=("s1", "cc", "s2", "d")):
    nc = bacc.Bacc(
        "TRN2", target_bir_lowering=False, debug=False,
        num_devices=cfg.NCORES,
    )
    D, T, CPC, CPG, SCB = cfg.D, cfg.TCALL, cfg.CPC, cfg.CPG, cfg.SCB
    TPB = T // 128            # token blocks per call (=8)
    GDT = BF16 if cfg.gdt == "bf16" else F32
    VCOLS = TPB if cfg.gdt == "bf16" else 2 * TPB
    MCOLS = T // 16 + VCOLS

    item_tab = nc.dram_tensor("item_tab", [cfg.I, D], GDT, kind="ExternalInput")
    g1_meta = nc.dram_tensor("g1_meta", [cfg.N1, 128, MCOLS], I16, kind="ExternalInput")
    g1_sidx = nc.dram_tensor("g1_sidx", [cfg.NG1, 128, cfg.SPG // 16], I16, kind="ExternalInput")
    g2_meta = nc.dram_tensor("g2_meta", [cfg.N2, 128, MCOLS], I16, kind="ExternalInput")
    g2_sidx = nc.dram_tensor("g2_sidx", [cfg.NG2, 128, cfg.SPG // 16], I16, kind="ExternalInput")
    u_w = nc.dram_tensor("u_w", [D, D], F32, kind="ExternalInput")
    i_w = nc.dram_tensor("i_w", [D, D], F32, kind="ExternalInput")
    ident = nc.dram_tensor("ident", [128, 128], F32, kind="ExternalInput")
    lhst = nc.dram_tensor("lhst", [8, 128, 128], GDT, kind="ExternalInput")

    ue_acc = nc.dram_tensor("ue_acc", [cfg.UPAD, D], F32, kind="ExternalOutput")
    ie_acc = nc.dram_tensor("ie_acc", [cfg.IPAD, D], F32, kind="ExternalOutput")
    user_out = nc.dram_tensor("user_out", [cfg.USH, D], F32, kind="ExternalOutput")
    item_out = nc.dram_tensor("item_out", [cfg.ISH, D], F32, kind="ExternalOutput")

    ue_cc_in = nc.dram_tensor("ue_cc_in", [cfg.USH, D], GDT)
    ue_full = nc.dram_tensor("ue_full", [cfg.U, D], GDT, addr_space="Shared")

    with tile.TileContext(nc) as tc:
        with (
            tc.tile_pool(name="consts", bufs=1) as cpool,
            tc.tile_pool(name="work", bufs=4) as pool,
            tc.tile_pool(name="gbuf", bufs=4) as gpool,
            tc.tile_pool(name="stg", bufs=2) as spool,
            tc.tile_pool(name="psum", bufs=4, space="PSUM") as psum,
            tc.tile_pool(name="psum3", bufs=2, space="PSUM") as psum3,
        ):
            lw = cpool.tile([128, 8, 128], GDT)
            nc.sync.dma_start(out=lw[:], in_=lhst.ap().rearrange("b k m -> k b m"))
            uw_t = cpool.tile([D, D], F32)
            nc.sync.dma_start(out=uw_t[:], in_=u_w.ap())
            iw_t = cpool.tile([D, D], F32)
            nc.sync.dma_start(out=iw_t[:], in_=i_w.ap())
            id_t = cpool.tile([128, 128], F32)
            nc.sync.dma_start(out=id_t[:], in_=ident.ap())

            def spmm(meta_d, sidx_d, tab_aps, ngroups_r, acc):
                grp = 0
                for tab_ap in tab_aps:
                    for _ in range(ngroups_r):
                        si = pool.tile([128, cfg.SPG // 16], I16, tag="si")
                        nc.sync.dma_start(out=si[:], in_=sidx_d.ap()[grp])
                        stage = spool.tile([128, SCB, D], F32, tag="stage")
                        for c in range(SCB):
                            call = grp * SCB + c
                            mt = pool.tile([128, MCOLS], I16, tag="mt")
                            nc.sync.dma_start(out=mt[:], in_=meta_d.ap()[call])
                            g = gpool.tile([128, TPB, D], GDT, tag="g")
                            nc.gpsimd.dma_gather(
                                out_ap=g[:], in_ap=tab_ap,
                                idxs_ap=mt[:, :T // 16],
                                num_idxs=T, num_idxs_reg=T, elem_size=D,
                            )
                            v = mt[:, T // 16:].bitcast(GDT)
                            vb = v.unsqueeze(2).broadcast_to([128, TPB, D])
                            m = gpool.tile([128, TPB, D], GDT, tag="m")
                            nc.vector.scalar_tensor_tensor(
                                out=m[:], in0=g[:], scalar=1.0, in1=vb,
                                op0=mybir.AluOpType.mult,
                                op1=mybir.AluOpType.mult,
                            )
                            pt = psum.tile([128, D], F32, tag="pt")
                            for b in range(TPB):
                                nc.tensor.matmul(
                                    pt[:], lw[:, b, :], m[:, b, :],
                                    start=(b == 0), stop=(b == TPB - 1),
                                )
                            nc.vector.tensor_copy(stage[:, c, :], pt[:])
                        nc.gpsimd.dma_scatter_add(
                            out_ap=acc.ap(), in_ap=stage[:], idxs_ap=si[:],
                            num_idxs=cfg.SPG, num_idxs_reg=cfg.SPG, elem_size=D,
                        )
                        grp += 1

            if "s1" in phases:
                tab1 = [
                    item_tab.ap()[r * cfg.RNG1:min((r + 1) * cfg.RNG1, cfg.I)]
                    for r in range(cfg.NRANGE1)
                ]
                spmm(g1_meta, g1_sidx, tab1, cfg.G1R, ue_acc)

            if "cc" in phases:
                nc.gpsimd.dma_start(out=ue_cc_in.ap(), in_=ue_acc.ap()[0:cfg.USH])
                nc.gpsimd.collective_compute(
                    "AllGather",
                    mybir.AluOpType.bypass,
                    ins=[ue_cc_in.ap().opt()],
                    outs=[ue_full.ap().opt()],
                    replica_groups=[list(range(cfg.NCORES))],
                )

            if "s2" in phases:
                tab2 = [
                    ue_full.ap()[r * cfg.RNG2:min((r + 1) * cfg.RNG2, cfg.U)]
                    for r in range(cfg.NRANGE2)
                ]
                spmm(g2_meta, g2_sidx, tab2, cfg.G2R, ie_acc)

            def dense_out(acc, w_tile, out_d, nrows, npad):
                for t in range(npad // 128):
                    xin = pool.tile([128, D], F32, tag="p3in")
                    nc.sync.dma_start(
                        out=xin[:], in_=acc.ap()[t * 128:(t + 1) * 128]
                    )
                    ptT = psum3.tile([128, D], F32, tag="p3T")
                    nc.tensor.transpose(ptT[:], xin[:], id_t[:])
                    xT = pool.tile([128, D], F32, tag="p3xT")
                    nc.vector.tensor_copy(xT[:], ptT[:])
                    ptZ = psum3.tile([128, D], F32, tag="p3Z")
                    nc.tensor.matmul(ptZ[:], xT[:], w_tile[:], start=True, stop=True)
                    o = pool.tile([128, D], F32, tag="p3o")
                    nc.scalar.activation(
                        out=o[:], in_=ptZ[:],
                        func=mybir.ActivationFunctionType.Sigmoid,
                    )
                    rows = min(128, nrows - t * 128)
                    if rows > 0:
                        nc.sync.dma_start(
                            out=out_d.ap()[t * 128:t * 128 + rows],
                            in_=o[:rows, :],
                        )

            if "d" in phases:
                dense_out(ue_acc, uw_t, user_out, cfg.USH, cfg.UPAD)
                dense_out(ie_acc, iw_t, item_out, cfg.ISH, cfg.IPAD)

    nc.compile()
    return nc


# ---------------------------------------------------------------------------
# entry point
# ---------------------------------------------------------------------------

TRACE = False
LAST_RESULT = {}


def kernel(**inputs):
    from concourse.bass_utils import run_bass_kernel_spmd

    cfg = FULL
    in_maps = prep_inputs(inputs, cfg)
    nc = build_program(cfg)
    br = run_bass_kernel_spmd(
        nc, in_maps, list(range(cfg.NCORES)), trace=TRACE,
    )
    LAST_RESULT["br"] = br
    res = br.results
    user_out = np.concatenate(
        [res[k]["user_out"] for k in range(cfg.NCORES)], axis=0
    )
    item_out = np.concatenate(
        [res[k]["item_out"] for k in range(cfg.NCORES)], axis=0
    )
    return (user_out, item_out)


# revision 21
# speedup vs baseline: 1.0665x; 1.0665x over previous
"""GCN layer (bipartite user-item SpMM x2 + dense + sigmoid) on 8 TRN2 NeuronCores.

Per core k:
  SpMM1 (ue = A @ item_emb), edges sharded by destination user-shard:
    host packs edges into fixed gather calls of TCALL=1024 tokens (the
    dma_gather HW path caps ~1024 indices per instruction); 8 consecutive
    tokens form a chunk with one destination row. SCB=7 calls form a
    scatter group; a destination row appears at most once per group (the
    CCE scatter-add is a read-modify-write, and duplicate rows inside one
    instruction race across the 16 DMA engines; groups are serialized by
    the Tile scheduler's WAW dependency on the accumulator).
    Device pipeline per call: dma_gather rows (bf16) -> scale by edge
    vals (DVE, bf16) -> chunk-sum via matmul with constant block-diagonal
    lhsT (PE, fp32 PSUM) -> stage; per group: dma_scatter_add partials
    into the HBM accumulator (CCE add, fp32).
    Gather tables are range-split to 25000 rows (indices are int16).
  AllGather ue (bf16) across the 8 cores.
  SpMM2 (ie = A^T @ ue): same pipeline, edges sharded by destination item.
  Dense: sigmoid(ue @ u_w), sigmoid(ie @ i_w) via PE transpose + matmul.
"""

import os
import sys
from dataclasses import dataclass

sys.path.insert(0, "/opt/trn_rl_repo")

import numpy as np

import concourse.bacc as bacc
import concourse.bass as bass
import concourse.mybir as mybir
import concourse.tile as tile


@dataclass(frozen=True)
class Cfg:
    U: int = 100000
    I: int = 50000
    D: int = 128
    NCORES: int = 8
    CHUNK: int = 8
    TCALL: int = 896           # tokens per gather call: one call is 57
                               # descriptors/lane, so two gathers + a scatter
                               # pipeline within the 128-desc/lane SWDGE ring
    SCB: int = 6               # calls per scatter group
    RNG1: int = 25000          # item-table range size (SpMM1 gathers)
    RNG2: int = 25000          # user-table range size (SpMM2 gathers)
    G1R: int = 49              # scatter groups per item range (spmm1)
    G2R: int = 25              # scatter groups per user range (spmm2)
    gdt: str = "bf16"          # gather/message dtype: "bf16" or "f32"

    @property
    def USH(self):
        return self.U // self.NCORES

    @property
    def ISH(self):
        return self.I // self.NCORES

    @property
    def NRANGE1(self):
        return (self.I + self.RNG1 - 1) // self.RNG1

    @property
    def NRANGE2(self):
        return (self.U + self.RNG2 - 1) // self.RNG2

    @property
    def NG1(self):
        return self.NRANGE1 * self.G1R      # scatter groups, spmm1

    @property
    def NG2(self):
        return self.NRANGE2 * self.G2R

    @property
    def N1(self):
        return self.NG1 * self.SCB          # gather calls, spmm1

    @property
    def N2(self):
        return self.NG2 * self.SCB

    @property
    def CPC(self):
        return self.TCALL // self.CHUNK     # chunks per call (=128)

    @property
    def CPG(self):
        return self.CPC * self.SCB          # chunk capacity per scatter group

    @property
    def SPG(self):
        return 128 * self.SCB               # scatter tokens per group (padded)

    @property
    def UPAD(self):
        return ((self.USH // 128) + 1) * 128

    @property
    def IPAD(self):
        return ((self.ISH // 128) + 1) * 128


FULL = Cfg()

F32 = mybir.dt.float32
BF16 = mybir.dt.bfloat16
I16 = mybir.dt.int16


# ---------------------------------------------------------------------------
# host-side packing
# ---------------------------------------------------------------------------

def _pack_phase(dst, src, val, dst_sh, src_rng, n_ranges, ngroups_r, cfg, dummy_row):
    """Pack one SpMM phase for one core.

    Returns:
      meta [NCALLS, 128, TCALL//16 + vcols] int16 — per call: gather
        indices (16-wrapped, 8x replicated) then bitcast vals (128-wrapped);
      sidx [NGROUPS, 128, CPG//16] int16 — per scatter group: chunk
        destination rows (16-wrapped, 8x replicated).
    """
    T, C, CPC, CPG, SCB = cfg.TCALL, cfg.CHUNK, cfg.CPC, cfg.CPG, cfg.SCB
    ncalls = n_ranges * ngroups_r * SCB
    ngroups = n_ranges * ngroups_r
    gidx_flat = np.zeros(ncalls * T, np.int16)
    vals_flat = np.zeros(ncalls * T, np.float32)
    SPG = cfg.SPG
    sidx_flat = np.full(ngroups * SPG, dummy_row, np.int16)

    for r in range(n_ranges):
        lo = r * src_rng
        m = (src >= lo) & (src < lo + src_rng)
        d = dst[m]
        s = (src[m] - lo).astype(np.int64)
        v = val[m]
        if d.size == 0:
            continue
        order = np.argsort(d, kind="stable")
        d, s, v = d[order], s[order], v[order]
        cnt = np.bincount(d, minlength=dst_sh)
        q = (cnt + C - 1) // C                      # chunks per dst row
        Q = int(q.sum())
        assert Q <= ngroups_r * CPG, (
            f"range {r}: {Q} chunks exceed budget {ngroups_r * CPG}"
        )
        chunk_off = np.cumsum(q) - q
        chunk_dst = np.repeat(np.arange(dst_sh, dtype=np.int64), q)
        chunk_j = np.arange(Q, dtype=np.int64) - chunk_off[chunk_dst]
        # color chunks -> groups; a dst row appears at most once per group
        color = (chunk_j + chunk_dst) % ngroups_r
        loads = np.bincount(color, minlength=ngroups_r)
        if (loads > CPG).any():
            color = _repair_colors(color, chunk_dst, ngroups_r, CPG)
            loads = np.bincount(color, minlength=ngroups_r)
        assert (loads <= CPG).all()
        order_c = np.argsort(color, kind="stable")
        slot_sorted = np.arange(Q) - (np.cumsum(loads) - loads)[color[order_c]]
        slot = np.empty(Q, np.int64)                # 0..CPG-1 within group
        slot[order_c] = slot_sorted
        grp_glob = r * ngroups_r + color
        call_in_grp = slot // CPC                   # 0..SCB-1
        prow = slot % CPC                           # psum row 0..127
        call_glob = grp_glob * SCB + call_in_grp
        # scatter token order within group: call*128 + psum_row (psum rows
        # CPC..127 are padding that scatters stale data into the dummy row)
        sidx_flat[grp_glob * SPG + call_in_grp * 128 + prow] = (
            chunk_dst.astype(np.int16)
        )
        # gather token position of chunk token t (0..7):
        #   i = (prow//16)*128 + (prow%16)*8 + t  within the call
        tok0 = call_glob * T + (prow // 16) * 128 + (prow % 16) * C
        ecnt_off = np.cumsum(cnt) - cnt
        within = np.arange(d.size) - ecnt_off[d]
        e_chunk = chunk_off[d] + within // C
        e_pos = tok0[e_chunk] + within % C
        gidx_flat[e_pos] = s.astype(np.int16)
        vals_flat[e_pos] = v

    gidx = _wrap16(gidx_flat.reshape(ncalls, T))                 # [NC,128,T/16]
    vnp = mybir.dt.np(BF16 if cfg.gdt == "bf16" else F32)
    vals = (
        vals_flat.reshape(ncalls, T // 128, 128)
        .transpose(0, 2, 1)
        .copy()
        .astype(vnp)
        .view(np.int16)
        .reshape(ncalls, 128, -1)
    )
    meta = np.concatenate([gidx, vals], axis=2)
    sidx = _wrap16(sidx_flat.reshape(ngroups, SPG))              # [NG,128,SPG/16]
    return meta, sidx


def _repair_colors(color, chunk_dst, ncolors, cap):
    color = color.copy()
    loads = np.bincount(color, minlength=ncolors)
    used = [set() for _ in range(ncolors)]
    for i in range(len(color)):
        used[color[i]].add(int(chunk_dst[i]))
    for c in np.where(loads > cap)[0]:
        over = np.where(color == c)[0]
        for i in over[cap:]:
            dsti = int(chunk_dst[i])
            for c2 in np.argsort(loads):
                if loads[c2] < cap and dsti not in used[c2]:
                    used[c].discard(dsti)
                    used[c2].add(dsti)
                    color[i] = c2
                    loads[c] -= 1
                    loads[c2] += 1
                    break
            else:
                raise AssertionError("color repair failed")
    return color


def _wrap16(a2d):
    """[n, m] -> [n, 128, m/16]: token i at partition i%16, slot i//16,
    replicated across the 8 gpsimd 16-partition groups."""
    n, m = a2d.shape
    w = a2d.reshape(n, m // 16, 16).transpose(0, 2, 1)
    return np.tile(w, (1, 8, 1)).copy()


def _chunk_lhst(cfg):
    """8 constant [128,128] matrices: block b maps token e to chunk 16*b+e//8."""
    C = cfg.CHUNK
    per = 128 // C
    lh = np.zeros((8, 128, 128), np.float32)
    for b in range(8):
        for e in range(128):
            lh[b, e, per * b + e // C] = 1.0
    return lh


def prep_inputs(inputs, cfg):
    rows = np.asarray(inputs["edge_rows"]).astype(np.int64)
    cols = np.asarray(inputs["edge_cols"]).astype(np.int64)
    vals = np.asarray(inputs["edge_vals"]).astype(np.float32)
    item = np.ascontiguousarray(np.asarray(inputs["item_embedding"], np.float32))
    u_w = np.ascontiguousarray(np.asarray(inputs["u_w"], np.float32))
    i_w = np.ascontiguousarray(np.asarray(inputs["i_w"], np.float32))
    gnp = mybir.dt.np(BF16 if cfg.gdt == "bf16" else F32)
    shared = {
        "item_tab": item.astype(gnp),
        "u_w": u_w,
        "i_w": i_w,
        "ident": np.eye(128, dtype=np.float32),
        "lhst": _chunk_lhst(cfg).astype(gnp),
    }
    in_maps = []
    ushard = rows // cfg.USH
    ishard = cols // cfg.ISH
    for k in range(cfg.NCORES):
        m1 = ushard == k
        meta1, sidx1 = _pack_phase(
            rows[m1] - k * cfg.USH, cols[m1], vals[m1], cfg.USH,
            cfg.RNG1, cfg.NRANGE1, cfg.G1R, cfg, cfg.UPAD - 1,
        )
        m2 = ishard == k
        # spmm2 gathers from 4 quarter-shard AllGather outputs: user u =
        # 12500*ku + j lives in table j//3125 at row ku*3125 + j%3125
        r2 = rows[m2]
        q4 = cfg.USH // 4
        ku, ju = r2 // cfg.USH, r2 % cfg.USH
        vrow = (ju // q4) * (cfg.NCORES * q4) + ku * q4 + (ju % q4)
        meta2, sidx2 = _pack_phase(
            cols[m2] - k * cfg.ISH, vrow, vals[m2], cfg.ISH,
            cfg.RNG2, cfg.NRANGE2, cfg.G2R, cfg, cfg.IPAD - 1,
        )
        in_maps.append({
            **shared,
            "g1_meta": meta1, "g1_sidx": sidx1,
            "g2_meta": meta2, "g2_sidx": sidx2,
        })
    return in_maps


# ---------------------------------------------------------------------------
# device program
# ---------------------------------------------------------------------------

def build_program(cfg, phases=("s1", "cc", "s2", "d")):
    nc = bacc.Bacc(
        "TRN2", target_bir_lowering=False, debug=False,
        num_devices=cfg.NCORES,
    )
    D, T, CPC, CPG, SCB = cfg.D, cfg.TCALL, cfg.CPC, cfg.CPG, cfg.SCB
    TPB = T // 128            # token blocks per call (=8)
    GDT = BF16 if cfg.gdt == "bf16" else F32
    VCOLS = TPB if cfg.gdt == "bf16" else 2 * TPB
    MCOLS = T // 16 + VCOLS

    item_tab = nc.dram_tensor("item_tab", [cfg.I, D], GDT, kind="ExternalInput")
    g1_meta = nc.dram_tensor("g1_meta", [cfg.N1, 128, MCOLS], I16, kind="ExternalInput")
    g1_sidx = nc.dram_tensor("g1_sidx", [cfg.NG1, 128, cfg.SPG // 16], I16, kind="ExternalInput")
    g2_meta = nc.dram_tensor("g2_meta", [cfg.N2, 128, MCOLS], I16, kind="ExternalInput")
    g2_sidx = nc.dram_tensor("g2_sidx", [cfg.NG2, 128, cfg.SPG // 16], I16, kind="ExternalInput")
    u_w = nc.dram_tensor("u_w", [D, D], F32, kind="ExternalInput")
    i_w = nc.dram_tensor("i_w", [D, D], F32, kind="ExternalInput")
    ident = nc.dram_tensor("ident", [128, 128], F32, kind="ExternalInput")
    lhst = nc.dram_tensor("lhst", [8, 128, 128], GDT, kind="ExternalInput")

    ue_acc = nc.dram_tensor("ue_acc", [cfg.UPAD, D], F32, kind="ExternalOutput")
    ie_acc = nc.dram_tensor("ie_acc", [cfg.IPAD, D], F32, kind="ExternalOutput")
    user_out = nc.dram_tensor("user_out", [cfg.USH, D], F32, kind="ExternalOutput")
    item_out = nc.dram_tensor("item_out", [cfg.ISH, D], F32, kind="ExternalOutput")

    Q4 = cfg.USH // 4
    ue_cc_in = [
        nc.dram_tensor(f"ue_cc_in{c}", [Q4, D], GDT) for c in range(4)
    ]
    ue_q = [
        nc.dram_tensor(f"ue_q{c}", [cfg.NCORES * Q4, D], GDT, addr_space="Shared")
        for c in range(4)
    ]

    with tile.TileContext(nc) as tc:
        with (
            tc.tile_pool(name="consts", bufs=1) as cpool,
            tc.tile_pool(name="work", bufs=4) as pool,
            tc.tile_pool(name="gbuf", bufs=4) as gpool,
            tc.tile_pool(name="stg", bufs=2) as spool,
            tc.tile_pool(name="psum", bufs=4, space="PSUM") as psum,
            tc.tile_pool(name="psum3", bufs=2, space="PSUM") as psum3,
        ):
            lw = cpool.tile([128, 8, 128], GDT)
            nc.sync.dma_start(out=lw[:], in_=lhst.ap().rearrange("b k m -> k b m"))
            uw_t = cpool.tile([D, D], F32)
            nc.sync.dma_start(out=uw_t[:], in_=u_w.ap())
            iw_t = cpool.tile([D, D], F32)
            nc.sync.dma_start(out=iw_t[:], in_=i_w.ap())
            id_t = cpool.tile([128, 128], F32)
            nc.sync.dma_start(out=id_t[:], in_=ident.ap())

            def spmm(meta_d, sidx_d, tab_aps, ngroups_r, acc):
                grp = 0
                for tab_ap in tab_aps:
                    for _ in range(ngroups_r):
                        si = pool.tile([128, cfg.SPG // 16], I16, tag="si")
                        nc.sync.dma_start(out=si[:], in_=sidx_d.ap()[grp])
                        stage = spool.tile([128, SCB, D], F32, tag="stage")
                        for c in range(SCB):
                            call = grp * SCB + c
                            mt = pool.tile([128, MCOLS], I16, tag="mt")
                            nc.sync.dma_start(out=mt[:], in_=meta_d.ap()[call])
                            g = gpool.tile([128, TPB, D], GDT, tag="g")
                            nc.gpsimd.dma_gather(
                                out_ap=g[:], in_ap=tab_ap,
                                idxs_ap=mt[:, :T // 16],
                                num_idxs=T, num_idxs_reg=T, elem_size=D,
                            )
                            v = mt[:, T // 16:].bitcast(GDT)
                            vb = v.unsqueeze(2).broadcast_to([128, TPB, D])
                            m = gpool.tile([128, TPB, D], GDT, tag="m")
                            nc.vector.scalar_tensor_tensor(
                                out=m[:], in0=g[:], scalar=1.0, in1=vb,
                                op0=mybir.AluOpType.mult,
                                op1=mybir.AluOpType.mult,
                            )
                            pt = psum.tile([128, D], F32, tag="pt")
                            for b in range(TPB):
                                nc.tensor.matmul(
                                    pt[:], lw[:, b, :], m[:, b, :],
                                    start=(b == 0), stop=(b == TPB - 1),
                                )
                            nc.vector.tensor_copy(stage[:, c, :], pt[:])
                        nc.gpsimd.dma_scatter_add(
                            out_ap=acc.ap(), in_ap=stage[:], idxs_ap=si[:],
                            num_idxs=cfg.SPG, num_idxs_reg=cfg.SPG, elem_size=D,
                        )
                        grp += 1

            if "s1" in phases:
                tab1 = [
                    item_tab.ap()[r * cfg.RNG1:min((r + 1) * cfg.RNG1, cfg.I)]
                    for r in range(cfg.NRANGE1)
                ]
                spmm(g1_meta, g1_sidx, tab1, cfg.G1R, ue_acc)

            if "cc" in phases:
                for c in range(4):
                    nc.gpsimd.dma_start(
                        out=ue_cc_in[c].ap(),
                        in_=ue_acc.ap()[c * Q4:(c + 1) * Q4],
                    )
                    nc.gpsimd.collective_compute(
                        "AllGather",
                        mybir.AluOpType.bypass,
                        ins=[ue_cc_in[c].ap().opt()],
                        outs=[ue_q[c].ap().opt()],
                        replica_groups=[list(range(cfg.NCORES))],
                    )

            if "s2" in phases:
                tab2 = [ue_q[c].ap() for c in range(4)]
                spmm(g2_meta, g2_sidx, tab2, cfg.G2R, ie_acc)

            def dense_out(acc, w_tile, out_d, nrows, npad):
                for t in range(npad // 128):
                    xin = pool.tile([128, D], F32, tag="p3in")
                    nc.sync.dma_start(
                        out=xin[:], in_=acc.ap()[t * 128:(t + 1) * 128]
                    )
                    ptT = psum3.tile([128, D], F32, tag="p3T")
                    nc.tensor.transpose(ptT[:], xin[:], id_t[:])
                    xT = pool.tile([128, D], F32, tag="p3xT")
                    nc.vector.tensor_copy(xT[:], ptT[:])
                    ptZ = psum3.tile([128, D], F32, tag="p3Z")
                    nc.tensor.matmul(ptZ[:], xT[:], w_tile[:], start=True, stop=True)
                    o = pool.tile([128, D], F32, tag="p3o")
                    nc.scalar.activation(
                        out=o[:], in_=ptZ[:],
                        func=mybir.ActivationFunctionType.Sigmoid,
                    )
                    rows = min(128, nrows - t * 128)
                    if rows > 0:
                        nc.sync.dma_start(
                            out=out_d.ap()[t * 128:t * 128 + rows],
                            in_=o[:rows, :],
                        )

            if "d" in phases:
                dense_out(ue_acc, uw_t, user_out, cfg.USH, cfg.UPAD)
                dense_out(ie_acc, iw_t, item_out, cfg.ISH, cfg.IPAD)

    nc.compile()
    return nc


# ---------------------------------------------------------------------------
# entry point
# ---------------------------------------------------------------------------

TRACE = False
LAST_RESULT = {}


def kernel(**inputs):
    from concourse.bass_utils import run_bass_kernel_spmd

    cfg = FULL
    in_maps = prep_inputs(inputs, cfg)
    nc = build_program(cfg)
    br = run_bass_kernel_spmd(
        nc, in_maps, list(range(cfg.NCORES)), trace=TRACE,
    )
    LAST_RESULT["br"] = br
    res = br.results
    user_out = np.concatenate(
        [res[k]["user_out"] for k in range(cfg.NCORES)], axis=0
    )
    item_out = np.concatenate(
        [res[k]["item_out"] for k in range(cfg.NCORES)], axis=0
    )
    return (user_out, item_out)
